# revision 10
# baseline (speedup 1.0000x reference)
# Fused conv3x3(same) + bias + tanh + x2 + stride-4 subsample, data-parallel
# over 8 NeuronCores.
#
# Math: out[b,oc,y,x] = 2*tanh(sum_{ic,ky,kx} w[oc,ic,ky,kx]*x[b,ic,4y+ky-1,4x+kx-1] + bias[oc])
# computed in fp16 like the reference. Since the spatial stride (4) exceeds the
# kernel size (3), every output pixel reads a disjoint 3x3x8 input patch, so the
# conv lowers exactly to a [72 -> 64] GEMM over 64*64 pixels per image. The host
# does the im2col rearrangement; each core runs the GEMM + bias + tanh for 4 of
# the 32 images. The trailing *2 and fp32 cast are applied on the host.
#
# The kernel is input-read-bound: the contraction rows live on SBUF partitions
# 0..79, and the partition->SDMA-engine map leaves 6 of the 16 engines without
# input work, capping HBM reads at ~260 GB/s. Two mitigations:
#   1. 44 of the 72 patch rows are stored in DRAM as fp8e4m3 and cast to fp16
#      in-flight by a gpsimd (SWDGE) DMA — the GEMM itself stays fp16, so only
#      the quantization of those rows' x values costs accuracy (measured
#      rel_err 1.5e-2 vs the 2e-2 budget; error scales with sqrt of the fp8
#      row fraction).
#   2. The fp8 block is padded to 64 partitions (rows 44..63 zero, 1B each) so
#      the fp8 DMA engages all 8 even engines evenly, and the fp16 block sits
#      on partitions 64..92 which map to the otherwise-idle odd engines.
#      Total per half-image: 64x2KB (fp8) + 29x4KB (fp16) = 244KB, ~16KB per
#      engine — balanced, vs v1's 64KB-on-even-engines for 320KB fp16.
# Contraction K = 93 (44 fp8 + 20 zero + 28 fp16 + bias row); matmul cost
# depends only on N, so the zero rows are free compute-wise.
#
# Pipeline: half-image stages (2048 pixels): 4 N=512 matmuls packed two-deep
# in PSUM partitions (chunk 2q+t -> partitions t*64:(t+1)*64 of bank q); the
# t=0/t=1 matmuls go to different PE column groups and execute concurrently.
# One 128-partition ACT computes tanh per half-image; output stores per half.
# The serial tanh chain (8 x ~1.11us on the Scalar engine) is the pacer, so
# matmuls running at the cold 1.2GHz PE clock (0.86us per stage) don't bind;
# only a token warmup burst remains (a long one would sit in the PE queue
# FIFO ahead of stage 0's matmuls and delay the whole chain).
import sys

import numpy as np

try:
    import concourse.bass as bass  # noqa: F401
except ImportError:
    sys.path.insert(0, "/opt/trn_rl_repo")

import ml_dtypes
import concourse.bass as bass  # noqa: F401
import concourse.bacc as bacc
import concourse.mybir as mybir
from concourse.bass_utils import run_bass_kernel_spmd

N_CORES = 8
B_FULL = 32
B_CORE = B_FULL // N_CORES  # 4 images per core
C_IN = 8
KH = KW = 3
K = C_IN * KH * KW  # 72 contraction
N8 = 44  # patch rows sent as fp8e4m3 (rest stay fp16)
KP8 = 64  # fp8 block padded to 64 partitions (rows N8..63 zero)
K16 = K - N8 + 1  # fp16 rows incl bias row = 29
KP = KP8 + K16  # 93 total contraction partitions
OC = 64
OH = OW = 64
NPIX = OH * OW  # 4096
HALF = NPIX // 2  # 2048
NH = 2 * B_CORE  # 8 half-image pipeline stages
N_WARM = 4
F16 = mybir.dt.float16
F8 = mybir.dt.float8e4
F32 = mybir.dt.float32

_PROGRAM = None


def build_program():
    from contextlib import ExitStack

    nc = bacc.Bacc("TRN2")
    # fp8 rows: one DMA per half-stage. Transfers must stay small: SDMA
    # engines round-robin queues at packet granularity and every DMA's
    # 16 completion sem-incs ride those queues, so a big SWDGE transfer
    # in flight stalls every HWDGE completion (and the sync queue's FIFO
    # triggers) behind its whole per-engine backlog.
    x8 = nc.dram_tensor("x8", [B_CORE, KP8, 2, HALF], F8, kind="ExternalInput")
    # fp16 rows: one DMA per half-stage
    x16 = nc.dram_tensor("x16", [B_CORE, K16, 2, HALF], F16, kind="ExternalInput")
    w = nc.dram_tensor("w", [KP, OC], F16, kind="ExternalInput")
    y = nc.dram_tensor("y", [NH, 2 * OC, HALF // 2], F16, kind="ExternalOutput")

    with ExitStack() as stack:
        w_tile = stack.enter_context(nc.sbuf_tensor([KP, OC], F16))
        # one buffer per half-image stage -> no buffer-reuse waits
        x_bufs = stack.enter_context(nc.sbuf_tensor([KP, NH, HALF], F16))
        a_bufs = stack.enter_context(nc.sbuf_tensor([2 * OC, NH, HALF // 2], F16))
        warm = stack.enter_context(nc.sbuf_tensor([2 * OC, 2 * OC], F16))
        # 8 banks of [128, 512]; stage i accumulates into banks 2i%8, 2i%8+1
        ps = stack.enter_context(nc.psum_tensor([2 * OC, 8, 512], F32))
        # Per-transfer input semaphores: concurrent DMAs complete out of
        # order, so one counting sem can't tell which transfer landed.
        sx8 = [stack.enter_context(nc.semaphore(f"s_x8{i}")) for i in range(NH)]
        sx = [stack.enter_context(nc.semaphore(f"s_x{i}")) for i in range(NH)]
        s_w = stack.enter_context(nc.semaphore("s_w"))
        s_warm = stack.enter_context(nc.semaphore("s_warm"))
        s_mm = stack.enter_context(nc.semaphore("s_mm"))
        s_act = stack.enter_context(nc.semaphore("s_act"))
        s_y = stack.enter_context(nc.semaphore("s_y"))
        block = stack.enter_context(nc.Block())

        @block.gpsimd
        def _(gpsimd):
            gpsimd.memset(warm[:], 0.0).then_inc(s_warm, 1)
            # fp8 -> fp16 casting DMAs (SWDGE is the only cast-capable path),
            # one per half-stage to bound the per-engine packet backlog
            for i in range(NH):
                gpsimd.dma_start(
                    out=x_bufs[:KP8, i, :], in_=x8[i // 2][:, i % 2, :]
                ).then_inc(sx8[i], 16)

        @block.sync
        def _(sync):
            # first half-image heads the critical path; w is tiny. The bias
            # rides in w row KP-1 (that patch row is constant 1.0).
            sync.dma_start(
                out=x_bufs[KP8:, 0, :], in_=x16[0][:, 0, :]
            ).then_inc(sx[0], 16)
            sync.dma_start(out=w_tile[:], in_=w[:]).then_inc(s_w, 16)
            for i in range(1, NH):
                sync.dma_start(
                    out=x_bufs[KP8:, i, :], in_=x16[i // 2][:, i % 2, :]
                ).then_inc(sx[i], 16)
            # output stores, paced by the ACT chain; the scalar queue must
            # not carry them (a trigger costs ~0.6us and would serialize
            # with the 1.1us ACTs)
            for i in range(NH):
                sync.wait_ge(s_act, i + 1)
                sync.dma_start(out=y[i], in_=a_bufs[:, i]).then_inc(s_y, 16)
            sync.wait_ge(s_y, 16 * NH)

        @block.tensor
        def _(tensor):
            tensor.wait_ge(s_warm, 1)
            for _ in range(N_WARM):
                nc.tensor.matmul(
                    ps[:OC, 7, :128],
                    warm[:, :OC],
                    warm[:],
                    start=True,
                    stop=True,
                )
            for i in range(NH):
                if i == 0:
                    tensor.wait_ge(s_w, 16)
                if i >= 4:
                    # psum bank pair reused; wait until ACT of stage i-4 read it
                    tensor.wait_ge(s_act, i - 3)
                tensor.wait_ge(sx8[i], 16)
                tensor.wait_ge(sx[i], 16)
                last = None
                for t in range(2):
                    for q in range(2):
                        c = 2 * q + t  # chunk within this half-image
                        last = nc.tensor.matmul(
                            ps[t * OC : (t + 1) * OC, (2 * i + q) % 8, :],
                            w_tile[:],
                            x_bufs[:, i, c * 512 : (c + 1) * 512],
                            start=True,
                            stop=True,
                        )
                last.then_inc(s_mm, 1)

        @block.scalar
        def _(scalar):
            for i in range(NH):
                scalar.wait_ge(s_mm, i + 1)
                bk = (2 * i) % 8
                nc.scalar.activation(
                    a_bufs[:, i],
                    ps[:, bk : bk + 2, :].rearrange("p b c -> p (b c)"),
                    mybir.ActivationFunctionType.Tanh,
                ).then_inc(s_act, 1)

    nc.finalize()
    return nc


def _get_program():
    global _PROGRAM
    if _PROGRAM is None:
        _PROGRAM = build_program()
    return _PROGRAM


def _im2col(x: np.ndarray) -> np.ndarray:
    """[B,8,256,256] fp32 -> [B,73,4096] fp16 patches, p=(ky*3+kx)*8+ic,
    row 72 = 1.0 (bias row; w row KP-1 carries the bias)."""
    B, C, H, W = x.shape
    xh = x.astype(np.float16)
    xpad = np.zeros((B, C, H + 2, W + 2), np.float16)
    xpad[:, :, 1 : H + 1, 1 : W + 1] = xh
    s = xpad.strides
    # windows[b,c,ky,kx,y,x] = xpad[b,c,4y+ky,4x+kx] = x[b,c,4y+ky-1,4x+kx-1]
    win = np.lib.stride_tricks.as_strided(
        xpad,
        shape=(B, C, KH, KW, OH, OW),
        strides=(s[0], s[1], s[2], s[3], 4 * s[2], 4 * s[3]),
    )
    out = np.empty((B, K + 1, NPIX), np.float16)
    np.copyto(
        out[:, :K].reshape(B, KH, KW, C, OH, OW), win.transpose(0, 2, 3, 1, 4, 5)
    )
    out[:, K] = np.float16(1.0)  # bias row
    return out


def run_sharded(x, weight, bias, **spmd_kwargs):
    """Returns (output, BassKernelResults). spmd_kwargs e.g. trace=True."""
    patches = _im2col(x)  # [32, 73, 4096] f16
    # fp8 block: rows 0..N8-1 quantized, rows N8..63 zero
    p8 = np.zeros((B_FULL, KP8, NPIX), ml_dtypes.float8_e4m3fn)
    p8[:, :N8] = patches[:, :N8].astype(ml_dtypes.float8_e4m3fn)
    p8 = p8.reshape(B_FULL, KP8, 2, HALF)
    # fp16 block: rows N8..71 + bias row
    p16 = np.ascontiguousarray(patches[:, N8:]).reshape(B_FULL, K16, 2, HALF)

    w_mat = np.zeros((KP, OC), np.float16)
    wk = weight.transpose(2, 3, 1, 0).reshape(K, OC).astype(np.float16)
    w_mat[:N8] = wk[:N8]
    w_mat[KP8 : KP8 + (K - N8)] = wk[N8:]
    w_mat[KP - 1] = bias.astype(np.float16).reshape(OC)

    in_maps = [
        {
            "x8": p8[c * B_CORE : (c + 1) * B_CORE],
            "x16": p16[c * B_CORE : (c + 1) * B_CORE],
            "w": w_mat,
        }
        for c in range(N_CORES)
    ]
    nc = _get_program()
    res = run_bass_kernel_spmd(nc, in_maps, list(range(N_CORES)), **spmd_kwargs)
    # y core shard: [8 half-stages, 128, 1024]; stage i = (image i//2, half
    # i%2); partition p = t*64+oc; column = q*512+col; pixel chunk = 4h+2q+t
    y16 = np.concatenate([r["y"] for r in res.results], axis=0)  # [64,128,1024]
    y16 = (
        y16.reshape(B_FULL, 2, 2, OC, 2, 512)  # [b, h, t, oc, q, col]
        .transpose(0, 3, 1, 4, 2, 5)  # [b, oc, h, q, t, col]
        .reshape(B_FULL, OC, NPIX)
    )
    # 2*tanh in fp16 then cast to fp32 == cast then *2 (exact: *2 is an
    # exponent bump, in-range for |tanh|<=1)
    out = y16.astype(np.float32).reshape(B_FULL, OC, OH, OW) * np.float32(2.0)
    return out, res


def kernel(x: np.ndarray, weight: np.ndarray, bias: np.ndarray) -> np.ndarray:
    return run_sharded(x, weight, bias)[0]


# revision 12
# speedup vs baseline: 2.2233x; 2.2233x over previous
# Fused conv3x3(same) + bias + tanh + x2 + stride-4 subsample, data-parallel
# over 8 NeuronCores.
#
# Math: out[b,oc,y,x] = 2*tanh(sum_{ic,ky,kx} w[oc,ic,ky,kx]*x[b,ic,4y+ky-1,4x+kx-1] + bias[oc])
# computed in fp16 like the reference. Since the spatial stride (4) exceeds the
# kernel size (3), every output pixel reads a disjoint 3x3x8 input patch, so the
# conv lowers exactly to a [72 -> 64] GEMM over 64*64 pixels per image. The host
# does the im2col rearrangement (pure data movement, fp16 cast is identical to
# the reference's .astype(float16)); each core runs the GEMM + bias + tanh for
# 4 of the 32 images. The trailing *2 and fp32 cast are exact in either order,
# so they are applied on the host after the fp16 tanh.
#
# Device kernel is hand-scheduled raw bacc (no Tile framework: avoids its
# multi-microsecond preamble/tail barriers). The pipeline works in half-images
# (2048 pixels): 4 N=512 matmuls packed two-deep in PSUM partitions (chunk
# 2q+t -> partitions t*64:(t+1)*64 of bank q) so one 128-partition ACT computes
# tanh per half and the output DMA engages all SBUF ports. Output DRAM layout
# is [B, 2, 64, 2048] (t = chunk parity); the host interleaves it back.
#
# The contraction is zero-padded 72 -> 80 rows: an 80-partition DMA spreads
# over all 16 SDMA engines (a 72-partition one only gets 12), which is worth
# more than the 11% extra bytes — the kernel is input-DMA-stream-bound.
# Per-descriptor runs are kept at 4 KiB (~17 GB/s per engine vs ~14 at 8 KiB).
import sys

import numpy as np

try:
    import concourse.bass as bass  # noqa: F401
except ImportError:
    sys.path.insert(0, "/opt/trn_rl_repo")

import concourse.bass as bass  # noqa: F401
import concourse.bacc as bacc
import concourse.mybir as mybir
from concourse.bass_utils import run_bass_kernel_spmd

N_CORES = 8
B_FULL = 32
B_CORE = B_FULL // N_CORES  # 4 images per core
C_IN = 8
KH = KW = 3
K = C_IN * KH * KW  # 72 contraction
KP = 80  # zero-padded contraction (16-SDMA-engine alignment)
OC = 64
OH = OW = 64
NPIX = OH * OW  # 4096
HALF = NPIX // 2  # 2048
NH = 2 * B_CORE  # 8 half-image pipeline stages
F16 = mybir.dt.float16
F32 = mybir.dt.float32

_PROGRAM = None


def build_program():
    from contextlib import ExitStack

    nc = bacc.Bacc("TRN2")
    xp = nc.dram_tensor("xp", [B_CORE, KP, 2, HALF], F16, kind="ExternalInput")
    w = nc.dram_tensor("w", [KP, OC], F16, kind="ExternalInput")
    y = nc.dram_tensor("y", [NH, 2 * OC, HALF // 2], F16, kind="ExternalOutput")

    with ExitStack() as stack:
        w_tile = stack.enter_context(nc.sbuf_tensor([KP, OC], F16))
        # one buffer per half-image stage -> no buffer-reuse waits; each DMA
        # writes one contiguous 4KiB run per partition
        x_bufs = stack.enter_context(nc.sbuf_tensor([KP, NH, HALF], F16))
        a_bufs = stack.enter_context(nc.sbuf_tensor([2 * OC, NH, HALF // 2], F16))
        warm = stack.enter_context(nc.sbuf_tensor([2 * OC, 2 * OC], F16))
        # 8 banks of [128, 512]; stage i accumulates into banks 2i%8, 2i%8+1
        ps = stack.enter_context(nc.psum_tensor([2 * OC, 8, 512], F32))
        # Per-stage input semaphores: concurrent DMAs complete out of order,
        # so one counting sem can't tell which transfer landed. s_y only
        # gates the final all-done wait, where order doesn't matter.
        sx = [stack.enter_context(nc.semaphore(f"s_x{i}")) for i in range(NH)]
        s_w = stack.enter_context(nc.semaphore("s_w"))
        s_warm = stack.enter_context(nc.semaphore("s_warm"))
        s_mm = stack.enter_context(nc.semaphore("s_mm"))
        s_act = stack.enter_context(nc.semaphore("s_act"))
        s_y = stack.enter_context(nc.semaphore("s_y"))
        block = stack.enter_context(nc.Block())

        @block.gpsimd
        def _(gpsimd):
            gpsimd.memset(warm[:], 0.0).then_inc(s_warm, 1)

        @block.sync
        def _(sync):
            # first half-image heads the critical path; w is tiny. The bias
            # rides in w row K (patch row K is constant 1.0), so there is no
            # separate bias operand anywhere.
            sync.dma_start(out=x_bufs[:, 0, :], in_=xp[0][:, 0, :]).then_inc(sx[0], 16)
            sync.dma_start(out=w_tile[:], in_=w[:]).then_inc(s_w, 16)
            for i in range(1, NH):
                sync.dma_start(
                    out=x_bufs[:, i, :], in_=xp[i // 2][:, i % 2, :]
                ).then_inc(sx[i], 16)
            # output stores, paced by the ACT chain; the scalar queue must
            # not carry them (a trigger costs ~0.6us and would serialize
            # with the 1.1us ACTs)
            for i in range(NH):
                sync.wait_ge(s_act, i + 1)
                sync.dma_start(out=y[i], in_=a_bufs[:, i]).then_inc(s_y, 16)
            sync.wait_ge(s_y, 16 * NH)

        @block.tensor
        def _(tensor):
            # The warmup burst is sized to end just as stage 0's patches land
            # (~2.8us after the PE queue starts): the warmups sit in the PE
            # queue FIFO ahead of the real matmuls, so more of them delays
            # stage 0 (v1's 50 pushed the first tanh out by ~3us), while
            # fewer (v3's 10) leaves the HAM activity window short of the
            # ~3.4us it needs to unthrottle the PE clock, and every matmul
            # then runs at 1.2GHz instead of 2.4.
            tensor.wait_ge(s_warm, 1)
            for _ in range(20):
                nc.tensor.matmul(
                    ps[:OC, 7, :128],
                    warm[:, :OC],
                    warm[:],
                    start=True,
                    stop=True,
                )
            for i in range(NH):
                if i == 0:
                    tensor.wait_ge(s_w, 16)
                if i >= 4:
                    # psum bank pair reused; wait until ACT of stage i-4 read it
                    tensor.wait_ge(s_act, i - 3)
                tensor.wait_ge(sx[i], 16)
                last = None
                for t in range(2):
                    for q in range(2):
                        c = 2 * q + t  # chunk within this half-image
                        last = nc.tensor.matmul(
                            ps[t * OC : (t + 1) * OC, (2 * i + q) % 8, :],
                            w_tile[:],
                            x_bufs[:, i, c * 512 : (c + 1) * 512],
                            start=True,
                            stop=True,
                        )
                last.then_inc(s_mm, 1)

        @block.scalar
        def _(scalar):
            for i in range(NH):
                scalar.wait_ge(s_mm, i + 1)
                bk = (2 * i) % 8
                nc.scalar.activation(
                    a_bufs[:, i],
                    ps[:, bk : bk + 2, :].rearrange("p b c -> p (b c)"),
                    mybir.ActivationFunctionType.Tanh,
                ).then_inc(s_act, 1)

    nc.finalize()
    return nc


def _get_program():
    global _PROGRAM
    if _PROGRAM is None:
        _PROGRAM = build_program()
    return _PROGRAM


def _im2col(x: np.ndarray) -> np.ndarray:
    """[B,8,256,256] fp32 -> [B,80,4096] fp16 patches, p=(ky*3+kx)*8+ic,
    rows 72..79 zero (padding for 16-SDMA-engine DMA spread)."""
    B, C, H, W = x.shape
    xh = x.astype(np.float16)
    xpad = np.zeros((B, C, H + 2, W + 2), np.float16)
    xpad[:, :, 1 : H + 1, 1 : W + 1] = xh
    s = xpad.strides
    # windows[b,c,ky,kx,y,x] = xpad[b,c,4y+ky,4x+kx] = x[b,c,4y+ky-1,4x+kx-1]
    win = np.lib.stride_tricks.as_strided(
        xpad,
        shape=(B, C, KH, KW, OH, OW),
        strides=(s[0], s[1], s[2], s[3], 4 * s[2], 4 * s[3]),
    )
    out = np.zeros((B, KP, NPIX), np.float16)
    np.copyto(
        out[:, :K].reshape(B, KH, KW, C, OH, OW), win.transpose(0, 2, 3, 1, 4, 5)
    )
    out[:, K] = np.float16(1.0)  # bias row: w row K carries the bias
    return out


def run_sharded(x, weight, bias, **spmd_kwargs):
    """Returns (output, BassKernelResults). spmd_kwargs e.g. trace=True."""
    patches = _im2col(x)  # [32, 80, 4096] f16, contiguous
    w_mat = np.zeros((KP, OC), np.float16)
    w_mat[:K] = weight.transpose(2, 3, 1, 0).reshape(K, OC).astype(np.float16)
    w_mat[K] = bias.astype(np.float16).reshape(OC)

    in_maps = [
        {
            "xp": patches[c * B_CORE : (c + 1) * B_CORE].reshape(B_CORE, KP, 2, HALF),
            "w": w_mat,
        }
        for c in range(N_CORES)
    ]
    nc = _get_program()
    res = run_bass_kernel_spmd(nc, in_maps, list(range(N_CORES)), **spmd_kwargs)
    # y core shard: [8 half-stages, 128, 1024]; stage i = (image i//2, half
    # i%2); partition p = t*64+oc; column = q*512+col; pixel chunk = 4h+2q+t
    y16 = np.concatenate([r["y"] for r in res.results], axis=0)  # [64,128,1024]
    y16 = (
        y16.reshape(B_FULL, 2, 2, OC, 2, 512)  # [b, h, t, oc, q, col]
        .transpose(0, 3, 1, 4, 2, 5)  # [b, oc, h, q, t, col]
        .reshape(B_FULL, OC, NPIX)
    )
    # 2*tanh in fp16 then cast to fp32 == cast then *2 (exact: *2 is an
    # exponent bump, in-range for |tanh|<=1)
    out = y16.astype(np.float32).reshape(B_FULL, OC, OH, OW) * np.float32(2.0)
    return out, res


def kernel(x: np.ndarray, weight: np.ndarray, bias: np.ndarray) -> np.ndarray:
    return run_sharded(x, weight, bias)[0]



# revision 13
# speedup vs baseline: 2.2983x; 1.0337x over previous
# Fused conv3x3(same) + bias + tanh + x2 + stride-4 subsample, data-parallel
# over 8 NeuronCores.
#
# Math: out[b,oc,y,x] = 2*tanh(sum_{ic,ky,kx} w[oc,ic,ky,kx]*x[b,ic,4y+ky-1,4x+kx-1] + bias[oc])
# computed in fp16 like the reference. Since the spatial stride (4) exceeds the
# kernel size (3), every output pixel reads a disjoint 3x3x8 input patch, so the
# conv lowers exactly to a [72 -> 64] GEMM over 64*64 pixels per image. The host
# does the im2col rearrangement (pure data movement, fp16 cast is identical to
# the reference's .astype(float16)); each core runs the GEMM + bias + tanh for
# 4 of the 32 images. The trailing *2 and fp32 cast are exact in either order,
# so they are applied on the host after the fp16 tanh.
#
# Device kernel is hand-scheduled raw bacc (no Tile framework: avoids its
# multi-microsecond preamble/tail barriers). The pipeline works in half-images
# (2048 pixels): 4 N=512 matmuls packed two-deep in PSUM partitions (chunk
# 2q+t -> partitions t*64:(t+1)*64 of bank q) so one 128-partition ACT computes
# tanh per half and the output DMA engages all SBUF ports. Output DRAM layout
# is [B, 2, 64, 2048] (t = chunk parity); the host interleaves it back.
#
# The contraction is zero-padded 72 -> 80 rows: an 80-partition DMA spreads
# over all 16 SDMA engines (a 72-partition one only gets 12), which is worth
# more than the 11% extra bytes — the kernel is input-DMA-stream-bound.
# Per-descriptor runs are kept at 4 KiB (~17 GB/s per engine vs ~14 at 8 KiB).
import sys

import numpy as np

try:
    import concourse.bass as bass  # noqa: F401
except ImportError:
    sys.path.insert(0, "/opt/trn_rl_repo")

import concourse.bass as bass  # noqa: F401
import concourse.bacc as bacc
import concourse.mybir as mybir
from concourse.bass_utils import run_bass_kernel_spmd

N_CORES = 8
B_FULL = 32
B_CORE = B_FULL // N_CORES  # 4 images per core
C_IN = 8
KH = KW = 3
K = C_IN * KH * KW  # 72 contraction
KP = 80  # zero-padded contraction (16-SDMA-engine alignment)
OC = 64
OH = OW = 64
NPIX = OH * OW  # 4096
HALF = NPIX // 2  # 2048
NH = 2 * B_CORE  # 8 half-image pipeline stages
F16 = mybir.dt.float16
F32 = mybir.dt.float32

_PROGRAM = None


def build_program():
    from contextlib import ExitStack

    nc = bacc.Bacc("TRN2")
    xp = nc.dram_tensor("xp", [B_CORE, KP, 2, HALF], F16, kind="ExternalInput")
    w = nc.dram_tensor("w", [KP, OC], F16, kind="ExternalInput")
    y = nc.dram_tensor("y", [NH, 2 * OC, HALF // 2], F16, kind="ExternalOutput")

    with ExitStack() as stack:
        w_tile = stack.enter_context(nc.sbuf_tensor([KP, OC], F16))
        # one buffer per half-image stage -> no buffer-reuse waits; each DMA
        # writes one contiguous 4KiB run per partition
        x_bufs = stack.enter_context(nc.sbuf_tensor([KP, NH, HALF], F16))
        a_bufs = stack.enter_context(nc.sbuf_tensor([2 * OC, NH, HALF // 2], F16))
        warm = stack.enter_context(nc.sbuf_tensor([2 * OC, 2 * OC], F16))
        # 8 banks of [128, 512]; stage i accumulates into banks 2i%8, 2i%8+1
        ps = stack.enter_context(nc.psum_tensor([2 * OC, 8, 512], F32))
        # Per-stage input semaphores: concurrent DMAs complete out of order,
        # so one counting sem can't tell which transfer landed. s_y only
        # gates the final all-done wait, where order doesn't matter.
        sx = [stack.enter_context(nc.semaphore(f"s_x{i}")) for i in range(NH)]
        s_w = stack.enter_context(nc.semaphore("s_w"))
        s_warm = stack.enter_context(nc.semaphore("s_warm"))
        s_mm = stack.enter_context(nc.semaphore("s_mm"))
        s_act = stack.enter_context(nc.semaphore("s_act"))
        s_y = stack.enter_context(nc.semaphore("s_y"))
        block = stack.enter_context(nc.Block())

        @block.gpsimd
        def _(gpsimd):
            gpsimd.memset(warm[:], 0.0).then_inc(s_warm, 1)

        @block.sync
        def _(sync):
            # first half-image heads the critical path; w is tiny. The bias
            # rides in w row K (patch row K is constant 1.0), so there is no
            # separate bias operand anywhere.
            sync.dma_start(out=x_bufs[:, 0, :], in_=xp[0][:, 0, :]).then_inc(sx[0], 16)
            sync.dma_start(out=w_tile[:], in_=w[:]).then_inc(s_w, 16)
            for i in range(1, NH):
                sync.dma_start(
                    out=x_bufs[:, i, :], in_=xp[i // 2][:, i % 2, :]
                ).then_inc(sx[i], 16)
            # output stores, paced by the ACT chain; the scalar queue must
            # not carry them (a trigger costs ~0.6us and would serialize
            # with the 1.1us ACTs)
            for i in range(NH):
                sync.wait_ge(s_act, i + 1)
                sync.dma_start(out=y[i], in_=a_bufs[:, i]).then_inc(s_y, 16)
            sync.wait_ge(s_y, 16 * NH)

        @block.tensor
        def _(tensor):
            # A short warmup burst bridges the PE-idle gap until stage 0's
            # patches land. The warmups sit in the PE queue FIFO ahead of the
            # real matmuls, so more of them DELAYS stage 0 (50 of them push
            # the first tanh out by ~3us); the input-paced matmul stream
            # never reaches the HAM clock-gate's ~3.4us sustained-busy
            # threshold either way, so the matmuls run at 1.2GHz regardless
            # and the only thing that matters is starting stage 0 early.
            tensor.wait_ge(s_warm, 1)
            for _ in range(10):
                nc.tensor.matmul(
                    ps[:OC, 7, :128],
                    warm[:, :OC],
                    warm[:],
                    start=True,
                    stop=True,
                )
            for i in range(NH):
                if i == 0:
                    tensor.wait_ge(s_w, 16)
                if i >= 4:
                    # psum bank pair reused; wait until ACT of stage i-4 read it
                    tensor.wait_ge(s_act, i - 3)
                tensor.wait_ge(sx[i], 16)
                last = None
                for t in range(2):
                    for q in range(2):
                        c = 2 * q + t  # chunk within this half-image
                        last = nc.tensor.matmul(
                            ps[t * OC : (t + 1) * OC, (2 * i + q) % 8, :],
                            w_tile[:],
                            x_bufs[:, i, c * 512 : (c + 1) * 512],
                            start=True,
                            stop=True,
                        )
                last.then_inc(s_mm, 1)

        @block.scalar
        def _(scalar):
            for i in range(NH):
                scalar.wait_ge(s_mm, i + 1)
                bk = (2 * i) % 8
                nc.scalar.activation(
                    a_bufs[:, i],
                    ps[:, bk : bk + 2, :].rearrange("p b c -> p (b c)"),
                    mybir.ActivationFunctionType.Tanh,
                ).then_inc(s_act, 1)

    nc.finalize()
    return nc


def _get_program():
    global _PROGRAM
    if _PROGRAM is None:
        _PROGRAM = build_program()
    return _PROGRAM


def _im2col(x: np.ndarray) -> np.ndarray:
    """[B,8,256,256] fp32 -> [B,80,4096] fp16 patches, p=(ky*3+kx)*8+ic,
    rows 72..79 zero (padding for 16-SDMA-engine DMA spread)."""
    B, C, H, W = x.shape
    xh = x.astype(np.float16)
    xpad = np.zeros((B, C, H + 2, W + 2), np.float16)
    xpad[:, :, 1 : H + 1, 1 : W + 1] = xh
    s = xpad.strides
    # windows[b,c,ky,kx,y,x] = xpad[b,c,4y+ky,4x+kx] = x[b,c,4y+ky-1,4x+kx-1]
    win = np.lib.stride_tricks.as_strided(
        xpad,
        shape=(B, C, KH, KW, OH, OW),
        strides=(s[0], s[1], s[2], s[3], 4 * s[2], 4 * s[3]),
    )
    out = np.zeros((B, KP, NPIX), np.float16)
    np.copyto(
        out[:, :K].reshape(B, KH, KW, C, OH, OW), win.transpose(0, 2, 3, 1, 4, 5)
    )
    out[:, K] = np.float16(1.0)  # bias row: w row K carries the bias
    return out


def run_sharded(x, weight, bias, **spmd_kwargs):
    """Returns (output, BassKernelResults). spmd_kwargs e.g. trace=True."""
    patches = _im2col(x)  # [32, 80, 4096] f16, contiguous
    w_mat = np.zeros((KP, OC), np.float16)
    w_mat[:K] = weight.transpose(2, 3, 1, 0).reshape(K, OC).astype(np.float16)
    w_mat[K] = bias.astype(np.float16).reshape(OC)

    in_maps = [
        {
            "xp": patches[c * B_CORE : (c + 1) * B_CORE].reshape(B_CORE, KP, 2, HALF),
            "w": w_mat,
        }
        for c in range(N_CORES)
    ]
    nc = _get_program()
    res = run_bass_kernel_spmd(nc, in_maps, list(range(N_CORES)), **spmd_kwargs)
    # y core shard: [8 half-stages, 128, 1024]; stage i = (image i//2, half
    # i%2); partition p = t*64+oc; column = q*512+col; pixel chunk = 4h+2q+t
    y16 = np.concatenate([r["y"] for r in res.results], axis=0)  # [64,128,1024]
    y16 = (
        y16.reshape(B_FULL, 2, 2, OC, 2, 512)  # [b, h, t, oc, q, col]
        .transpose(0, 3, 1, 4, 2, 5)  # [b, oc, h, q, t, col]
        .reshape(B_FULL, OC, NPIX)
    )
    # 2*tanh in fp16 then cast to fp32 == cast then *2 (exact: *2 is an
    # exponent bump, in-range for |tanh|<=1)
    out = y16.astype(np.float32).reshape(B_FULL, OC, OH, OW) * np.float32(2.0)
    return out, res


def kernel(x: np.ndarray, weight: np.ndarray, bias: np.ndarray) -> np.ndarray:
    return run_sharded(x, weight, bias)[0]

